# revision 20
# baseline (speedup 1.0000x reference)
"""DeepFRI GCN (3x GCNConv -> concat -> mean-pool -> 3-layer FC head) on 8 trn2
NeuronCores.

Sharding: graph/data parallel. Core c owns graphs [8c, 8c+8) and their node
slice (batch is sorted, so slices are contiguous). Weights are replicated.

Math (per GCN layer, exploiting  dinv[s]*dinv[d] edge norm):
    z  = h_prev @ W            (dense, feat-major via PE)
    z' = dinv[:,None] * z      (node-major)
    h  = dinv[:,None] * (segment_sum(z'[src], dst) + z') + b
Per-edge gather of z'[src] spans all cores -> AllGather z' each layer, then
indirect-DMA gather 128 rows/chunk and scatter-add via a selection-matrix
matmul into a PSUM accumulator per 128-node dst block.
"""

import math

import numpy as np

from concourse import bacc, bass, mybir
from concourse.bass_utils import run_bass_kernel_spmd
from concourse.masks import make_identity
from concourse.tile import TileContext

# ---- problem constants (hardcoded; kernel.py must be self-contained) ----
N = 50000
E = 800000
IN_DIM = 1280
D = 512                   # GCN hidden dim (all 3 layers)
N_GRAPHS = 64
NCORES = 8
GPC = N_GRAPHS // NCORES  # graphs per core
FC0, FC1, OUT = 1024, 512, 256
P = 128
F32 = mybir.dt.float32
F16 = mybir.dt.float16
I32 = mybir.dt.int32

_cache = {}


# --------------------------------------------------------------------------
# host-side layout prep (integer/index work only; all float math on device)
# --------------------------------------------------------------------------

def _host_prep(x, edge_index, batch):
    x = np.asarray(x, dtype=np.float32)
    src = np.asarray(edge_index[0], dtype=np.int64)
    dst = np.asarray(edge_index[1], dtype=np.int64)
    batch = np.asarray(batch, dtype=np.int64)
    n = x.shape[0]

    deg = np.bincount(dst, minlength=n)              # in-degree (no self loop)

    # graph-aligned node shards
    bounds = np.searchsorted(batch, np.arange(0, N_GRAPHS + 1, GPC))
    n_lo, n_hi = bounds[:-1], bounds[1:]
    n_per = n_hi - n_lo
    pad_n = int(math.ceil(n_per.max() / 512) * 512)
    nb = pad_n // P

    owner = batch // GPC                             # owning core of each node
    pad_gid = owner * pad_n + (np.arange(n) - n_lo[owner])  # padded global id

    e_owner = batch[dst] // GPC
    src_half = batch[src] // GPC >= NCORES // 2    # which half-table has src
    per_core = []
    cnts = np.zeros((NCORES, nb, 2), dtype=np.int64)
    for c in range(NCORES):
        m = e_owner == c
        s_c, d_c, h_c = src[m], dst[m], src_half[m].astype(np.int64)
        blk = (d_c - n_lo[c]) >> 7
        order = np.argsort(blk * 2 + h_c, kind="stable")
        s_c, d_c, h_c = s_c[order], d_c[order], h_c[order]
        blk = blk[order]
        cnts[c] = np.bincount(blk * 2 + h_c, minlength=2 * nb).reshape(nb, 2)
        per_core.append((s_c, d_c, blk, h_c))

    # per-(block, half) chunk counts, shared across cores
    sched2 = (-(-cnts.max(axis=0) // P)).astype(np.int64)       # [nb, 2]
    zero = sched2.sum(axis=1) == 0
    sched2[zero, 0] = 1
    nch = int(sched2.sum())
    base2 = np.concatenate([[0], np.cumsum(sched2.reshape(-1))])[:-1].reshape(
        nb, 2)

    half_off = (NCORES // 2) * pad_n
    in_maps = []
    for c in range(NCORES):
        s_c, d_c, blk, h_c = per_core[c]
        lo = n_lo[c]
        cnt = n_hi[c] - lo

        srcW = np.zeros((16, nch * 8), dtype=np.int16)   # wrapped idx layout
        dstT = np.full((P, nch), -1.0, dtype=np.float32)
        estart = np.concatenate(
            [[0], np.cumsum(cnts[c].reshape(-1))]).reshape(-1)
        for b in range(nb):
            for h in range(2):
                e0 = estart[b * 2 + h]
                k = cnts[c, b, h]
                if k == 0:
                    continue
                eb = slice(e0, e0 + k)
                j = np.arange(k)
                cc, pp = np.divmod(j, P)
                ci = base2[b, h] + cc
                idx16 = (pad_gid[s_c[eb]] - h * half_off).astype(np.int16)
                srcW[j % 16, ci * 8 + (j % P) // 16] = idx16
                dstT[pp, ci] = (d_c[eb] - lo - (b << 7)).astype(np.float32)
        srcT = np.tile(srcW, (8, 1))                     # replicate to 128 part

        assert (NCORES // 2) * pad_n < 32768, "int16 gather index overflow"
        xT = np.zeros((IN_DIM, pad_n), dtype=np.float16)
        xT[:, :cnt] = x[lo:n_hi[c]].T.astype(np.float16)

        deg_pad = np.zeros(pad_n, dtype=np.float32)
        deg_pad[:cnt] = deg[lo:n_hi[c]]
        degT = deg_pad.reshape(nb, P).T.copy()

        poolT = np.zeros((P, GPC * nb), dtype=np.float32)
        bloc = batch[lo:n_hi[c]] - c * GPC            # local graph id in [0,GPC)
        idx = np.arange(cnt)
        poolT[idx % P, (idx // P) * GPC + bloc] = 1.0

        iotaT = np.tile(np.arange(P, dtype=np.float32)[None, :], (P, 1))

        in_maps.append({
            "xT": xT, "srcT": srcT, "dstT": dstT, "degT": degT,
            "poolT": poolT, "iotaT": iotaT,
        })
    return in_maps, pad_n, tuple(map(tuple, sched2.tolist()))


def _weight_maps(kw):
    """Shared (replicated) weight arrays, reshaped for the device layout."""
    out = {
        "wg1": np.asarray(kw["Wg1"], np.float16),
        "wg2": np.asarray(kw["Wg2"], np.float16),
        "wg3": np.asarray(kw["Wg3"], np.float16),
        "wr": np.asarray(kw["Wr"], np.float32),
        "wf": np.asarray(kw["Wf"], np.float32),
        "wo": np.asarray(kw["Wo"], np.float32),
    }
    for i in (1, 2, 3):
        out[f"bgr{i}"] = np.tile(np.asarray(kw[f"bg{i}"], np.float32)[None, :],
                                 (P, 1))
    out["brT"] = np.asarray(kw["br"], np.float32).reshape(FC0 // P, P).T.copy()
    out["bfT"] = np.asarray(kw["bf"], np.float32).reshape(FC1 // P, P).T.copy()
    out["boT"] = np.asarray(kw["bo"], np.float32).reshape(OUT // P, P).T.copy()
    return out


# --------------------------------------------------------------------------
# device program
# --------------------------------------------------------------------------

def _build(pad_n, sched):
    sched2 = np.asarray(sched, dtype=np.int64)          # [nb, 2]
    nb = pad_n // P
    ngrp = pad_n // 512
    nch = int(sched2.sum())
    base2 = np.concatenate(
        [[0], np.cumsum(sched2.reshape(-1))])[:-1].reshape(nb, 2).astype(int)
    half_off = (NCORES // 2) * pad_n

    nc = bacc.Bacc()
    dp = nc.declare_dram_parameter
    xT = dp("xT", [IN_DIM, pad_n], F16, isOutput=False)
    wg = [None,
          dp("wg1", [IN_DIM, D], F16, isOutput=False),
          dp("wg2", [D, D], F16, isOutput=False),
          dp("wg3", [D, D], F16, isOutput=False)]
    bgr = [None,
           dp("bgr1", [P, D], F32, isOutput=False),
           dp("bgr2", [P, D], F32, isOutput=False),
           dp("bgr3", [P, D], F32, isOutput=False)]
    wr = dp("wr", [3 * D, FC0], F32, isOutput=False)
    wf = dp("wf", [FC0, FC1], F32, isOutput=False)
    wo = dp("wo", [FC1, OUT], F32, isOutput=False)
    brT = dp("brT", [P, FC0 // P], F32, isOutput=False)
    bfT = dp("bfT", [P, FC1 // P], F32, isOutput=False)
    boT = dp("boT", [P, OUT // P], F32, isOutput=False)
    degT = dp("degT", [P, nb], F32, isOutput=False)
    poolT = dp("poolT", [P, GPC * nb], F32, isOutput=False)
    iotaT = dp("iotaT", [P, P], F32, isOutput=False)
    srcT = dp("srcT", [P, nch * 8], mybir.dt.int16, isOutput=False)
    dstT = dp("dstT", [P, nch], F32, isOutput=False)
    out = dp("out", [GPC, OUT], F32, isOutput=True)

    zp_loc = nc.dram_tensor("zp_loc", [pad_n, D], F16)
    zp_full = nc.dram_tensor("zp_full", [NCORES * pad_n, D], F16,
                             addr_space="Shared")
    hloc = [None, nc.dram_tensor("h_a", [pad_n, D], F16),
            nc.dram_tensor("h_b", [pad_n, D], F16)]

    rg = [list(range(NCORES))]

    with TileContext(nc) as tc:
        with (
            tc.tile_pool(name="cpool", bufs=1) as cpool,
            tc.tile_pool(name="wpool", bufs=1) as wpool,
            tc.tile_pool(name="work", bufs=3) as work,
            tc.tile_pool(name="srcin", bufs=14) as spool,
            tc.tile_pool(name="zpool", bufs=8) as zpool,
            tc.tile_pool(name="gath", bufs=3) as gpool,
            tc.tile_pool(name="selp", bufs=8) as selpool,
            tc.tile_pool(name="psA", bufs=2, space="PSUM") as psA,
            tc.tile_pool(name="psT", bufs=2, space="PSUM") as psT,
            tc.tile_pool(name="psG", bufs=2, space="PSUM") as psG,
            tc.tile_pool(name="psP", bufs=1, space="PSUM") as psP,
        ):
            # ---- prologue: constants ----
            deg_sb = work.tile([P, nb], F32, tag="deg")
            nc.sync.dma_start(out=deg_sb[:], in_=degT[:])
            sq_sb = work.tile([P, nb], F32, tag="sq")
            nc.scalar.activation(sq_sb[:], deg_sb[:],
                                 mybir.ActivationFunctionType.Sqrt, bias=1.0)
            dinv = cpool.tile([P, nb], F32, tag="dinv")
            nc.vector.reciprocal(dinv[:], sq_sb[:])

            iota = cpool.tile([P, P], F32, tag="iota")
            nc.sync.dma_start(out=iota[:], in_=iotaT[:])
            ident = cpool.tile([P, P], F32, tag="ident")
            make_identity(nc, ident[:])
            pool_sb = cpool.tile([P, GPC * nb], F32, tag="pool")
            nc.sync.dma_start(out=pool_sb[:], in_=poolT[:])
            ones = cpool.tile([P, 1], F32, tag="ones")
            nc.vector.memset(ones[:], 1.0)
            src_sb = cpool.tile([P, nch * 8], mybir.dt.int16, tag="srci")
            nc.sync.dma_start(out=src_sb[:], in_=srcT[:])
            dstv_sb = cpool.tile([P, nch], F32, tag="dstv")
            nc.sync.dma_start(out=dstv_sb[:], in_=dstT[:])
            bgr_sb = [None]
            for l in (1, 2, 3):
                t = cpool.tile([P, D], F32, tag=f"bgr{l}")
                nc.sync.dma_start(out=t[:], in_=bgr[l][:])
                bgr_sb.append(t)
            brT_sb = cpool.tile([P, FC0 // P], F32, tag="brT")
            nc.sync.dma_start(out=brT_sb[:], in_=brT[:])
            bfT_sb = cpool.tile([P, FC1 // P], F32, tag="bfT")
            nc.sync.dma_start(out=bfT_sb[:], in_=bfT[:])
            boT_sb = cpool.tile([P, OUT // P], F32, tag="boT")
            nc.sync.dma_start(out=boT_sb[:], in_=boT[:])

            gcat = cpool.tile([GPC, 3 * D], F32, tag="gcat")
            rec_sb = cpool.tile([GPC, 1], F32, tag="rec")

            # ---- 3 GCN layers ----
            for l in (1, 2, 3):
                din = IN_DIM if l == 1 else D
                wk = din // P
                srcmajor = xT if l == 1 else hloc[l - 1]

                # layer weights resident: wall[:, (k*4+f)*P : ...] = W[kP:,fP:]
                wall = wpool.tile([P, wk * D], F16, tag=f"wall{l}")
                for k in range(wk):
                    for f in range(4):
                        nc.sync.dma_start(
                            out=wall[:, (k * 4 + f) * P:(k * 4 + f + 1) * P],
                            in_=wg[l][k * P:(k + 1) * P, f * P:(f + 1) * P])

                # dense: z^T (feat-major) -> scale by dinv -> zp_loc (node-major)
                for g in range(ngrp):
                    sts = []
                    for k in range(wk):
                        st = spool.tile([P, 512], F16, tag="srcin",
                                        name=f"st{k}")
                        if l == 1:
                            nc.sync.dma_start(
                                out=st[:],
                                in_=srcmajor[k * P:(k + 1) * P,
                                             g * 512:(g + 1) * 512])
                        else:
                            nc.sync.dma_start(
                                out=st[:],
                                in_=srcmajor[g * 512:(g + 1) * 512,
                                             k * P:(k + 1) * P],
                                transpose=True)
                        sts.append(st)
                    ztsb = []
                    for f in range(4):
                        ztf = psA.tile([P, 512], F32, tag="zt")
                        for k in range(wk):
                            nc.tensor.matmul(
                                ztf[:],
                                lhsT=wall[:, (k * 4 + f) * P:(k * 4 + f + 1) * P],
                                rhs=sts[k][:],
                                start=(k == 0), stop=(k == wk - 1))
                        t = zpool.tile([P, 512], F32, tag="ztsb")
                        nc.vector.tensor_copy(t[:], ztf[:])
                        ztsb.append(t)
                    for j in range(4):
                        b = g * 4 + j
                        zp_sb = work.tile([P, D], F16, tag="zpsb")
                        for f in range(4):
                            ptr = psT.tile([P, P], F32, tag="tr")
                            nc.tensor.transpose(
                                ptr[:], ztsb[f][:, j * P:(j + 1) * P], ident[:])
                            nc.vector.tensor_scalar_mul(
                                zp_sb[:, f * P:(f + 1) * P], ptr[:],
                                dinv[:, b:b + 1])
                        nc.sync.dma_start(out=zp_loc[b * P:(b + 1) * P, :],
                                          in_=zp_sb[:])

                # all-gather z' across cores
                nc.gpsimd.collective_compute(
                    "AllGather", mybir.AluOpType.bypass, replica_groups=rg,
                    ins=[zp_loc[:]], outs=[zp_full[:]])

                # aggregation per 128-node dst block
                pool_ps = psP.tile([GPC, D], F32, tag="poolps")
                if l == 1:
                    cnt_ps = psP.tile([GPC, 1], F32, tag="cntps")
                for b in range(nb):
                    agg = psG.tile([P, D], F32, tag="agg")
                    tot = int(sched2[b].sum())
                    done = 0
                    for h in range(2):
                        nchunk = int(sched2[b, h])
                        if nchunk == 0:
                            continue
                        cb = base2[b, h]
                        gt = gpool.tile([P, nchunk * D], F16, tag="g",
                                        name=f"g{h}")
                        nc.gpsimd.dma_gather(
                            out_ap=gt[:].rearrange("p (c d) -> p c d", d=D),
                            in_ap=(zp_full[:half_off, :] if h == 0
                                   else zp_full[half_off:, :]),
                            idxs_ap=src_sb[:, cb * 8:(cb + nchunk) * 8],
                            num_idxs=nchunk * P, num_idxs_reg=nchunk * P,
                            elem_size=D, single_packet=False)
                        for c in range(nchunk):
                            ci = cb + c
                            sel = selpool.tile([P, P], F16, tag="sel")
                            nc.vector.tensor_tensor(
                                sel[:],
                                dstv_sb[:, ci:ci + 1].to_broadcast([P, P]),
                                iota[:], op=mybir.AluOpType.is_equal)
                            nc.tensor.matmul(
                                agg[:], lhsT=sel[:],
                                rhs=gt[:, c * D:(c + 1) * D],
                                start=(done == 0), stop=(done == tot - 1))
                            done += 1
                    zpb = work.tile([P, D], F16, tag="zpb")
                    nc.sync.dma_start(out=zpb[:],
                                      in_=zp_loc[b * P:(b + 1) * P, :])
                    sum_sb = work.tile([P, D], F32, tag="sum")
                    nc.vector.tensor_add(sum_sb[:], agg[:], zpb[:])
                    h_sb = work.tile([P, D], F32, tag="h")
                    nc.vector.scalar_tensor_tensor(
                        out=h_sb[:], in0=sum_sb[:], scalar=dinv[:, b:b + 1],
                        in1=bgr_sb[l][:], op0=mybir.AluOpType.mult,
                        op1=mybir.AluOpType.add)
                    nc.tensor.matmul(
                        pool_ps[:], lhsT=pool_sb[:, b * GPC:(b + 1) * GPC],
                        rhs=h_sb[:], start=(b == 0), stop=(b == nb - 1),
                        skip_group_check=True)
                    if l == 1:
                        nc.tensor.matmul(
                            cnt_ps[:], lhsT=pool_sb[:, b * GPC:(b + 1) * GPC],
                            rhs=ones[:], start=(b == 0), stop=(b == nb - 1),
                            skip_group_check=True)
                    if l < 3:
                        h16 = work.tile([P, D], F16, tag="h16")
                        nc.vector.tensor_copy(h16[:], h_sb[:])
                        nc.sync.dma_start(
                            out=hloc[l][b * P:(b + 1) * P, :], in_=h16[:])

                # epilogue: mean-pool -> gcat columns
                if l == 1:
                    cnt_sb = work.tile([GPC, 1], F32, tag="cnt")
                    nc.vector.tensor_scalar_max(cnt_sb[:], cnt_ps[:], 1.0)
                    nc.vector.reciprocal(rec_sb[:], cnt_sb[:])
                nc.vector.tensor_scalar_mul(
                    gcat[:, (l - 1) * D:l * D], pool_ps[:], rec_sb[:])

            # ---- FC head (per-core GPC graphs) ----
            gT = cpool.tile([P, 12 * GPC], F32, tag="gT")
            for k in range(12):
                ptr = psT.tile([P, P], F32, tag="tr")
                nc.tensor.transpose(ptr[:, :GPC],
                                    gcat[:, k * P:(k + 1) * P],
                                    ident[:GPC, :GPC])
                nc.vector.tensor_copy(gT[:, k * GPC:(k + 1) * GPC],
                                      ptr[:, :GPC])

            def fc(in_sb, w_dram, kin, fout, bias_sb, act, tagp):
                out_sb = cpool.tile([P, fout * GPC], F32, tag=f"{tagp}o")
                for f in range(fout):
                    ps = psT.tile([P, P], F32, tag="tr")
                    for k in range(kin):
                        wt = spool.tile([P, P], F32, tag="hw")
                        nc.sync.dma_start(
                            out=wt[:],
                            in_=w_dram[k * P:(k + 1) * P, f * P:(f + 1) * P])
                        nc.tensor.matmul(ps[:, :GPC], lhsT=wt[:],
                                         rhs=in_sb[:, k * GPC:(k + 1) * GPC],
                                         start=(k == 0), stop=(k == kin - 1))
                    nc.scalar.activation(out_sb[:, f * GPC:(f + 1) * GPC],
                                         ps[:, :GPC], act,
                                         bias=bias_sb[:, f:f + 1])
                return out_sb

            relu = mybir.ActivationFunctionType.Relu
            iden = mybir.ActivationFunctionType.Identity
            fc1 = fc(gT, wr, 12, FC0 // P, brT_sb, relu, "fc1")
            fc2 = fc(fc1, wf, FC0 // P, FC1 // P, bfT_sb, relu, "fc2")
            fc3 = fc(fc2, wo, FC1 // P, OUT // P, boT_sb, iden, "fc3")

            out_sb = work.tile([GPC, OUT], F32, tag="outsb")
            for f in range(OUT // P):
                ptr = psT.tile([P, P], F32, tag="tr")
                nc.tensor.transpose(ptr[:GPC, :],
                                    fc3[:, f * GPC:(f + 1) * GPC], ident[:])
                nc.vector.tensor_copy(out_sb[:, f * P:(f + 1) * P],
                                      ptr[:GPC, :])
            nc.sync.dma_start(out=out[:], in_=out_sb[:])

    nc.finalize()
    return nc


# --------------------------------------------------------------------------
# PJRT runner (mirrors bass2jax.run_bass_via_pjrt, but reusable + timeable)
# --------------------------------------------------------------------------

def _make_runner(nc):
    import jax
    from jax.experimental.shard_map import shard_map
    from jax.sharding import Mesh, NamedSharding, PartitionSpec

    from concourse import bass2jax

    bass2jax.install_neuronx_cc_hook()
    pname = nc.partition_id_tensor.name if nc.partition_id_tensor else None
    in_names, out_names, out_avals = [], [], []
    for alloc in nc.m.functions[0].allocations:
        if not isinstance(alloc, mybir.MemoryLocationSet):
            continue
        name = alloc.memorylocations[0].name
        if alloc.kind == "ExternalInput":
            if name != pname:
                in_names.append(name)
        elif alloc.kind == "ExternalOutput":
            out_names.append(name)
            out_avals.append(jax.core.ShapedArray(
                tuple(alloc.tensor_shape), mybir.dt.np(alloc.dtype)))
    n_params = len(in_names)
    all_names = in_names + out_names + ([pname] if pname else [])

    def _body(*args):
        operands = list(args)
        if pname:
            operands.append(bass2jax.partition_id_tensor())
        return tuple(bass2jax._bass_exec_p.bind(
            *operands, out_avals=tuple(out_avals), in_names=tuple(all_names),
            out_names=tuple(out_names), lowering_input_output_aliases=(),
            sim_require_finite=True, sim_require_nnan=True, nc=nc))

    devices = jax.devices()[:NCORES]
    mesh = Mesh(np.asarray(devices), ("core",))
    n_outs = len(out_names)
    sharded = jax.jit(
        shard_map(_body, mesh=mesh,
                  in_specs=(PartitionSpec("core"),) * (n_params + n_outs),
                  out_specs=(PartitionSpec("core"),) * n_outs,
                  check_rep=False),
        keep_unused=True)
    sharding = NamedSharding(mesh, PartitionSpec("core"))
    return sharded, in_names[:n_params], out_names, out_avals, sharding


def _run(nc, in_maps, time_iters=0):
    import jax
    import time as _time

    sharded, in_names, out_names, out_avals, sharding = _make_runner(nc)
    concat_in = [np.concatenate([np.asarray(m[nm]) for m in in_maps], axis=0)
                 for nm in in_names]
    concat_zeros = [np.zeros((NCORES * a.shape[0], *a.shape[1:]), a.dtype)
                    for a in out_avals]
    args = [jax.device_put(a, sharding) for a in concat_in + concat_zeros]
    out_arrs = sharded(*args)
    jax.block_until_ready(out_arrs)
    best = None
    for _ in range(time_iters):
        t0 = _time.perf_counter()
        o = sharded(*args)
        jax.block_until_ready(o)
        dt = _time.perf_counter() - t0
        best = dt if best is None else min(best, dt)
    results = [
        {nm: np.asarray(out_arrs[i]).reshape(NCORES, *out_avals[i].shape)[c]
         for i, nm in enumerate(out_names)}
        for c in range(NCORES)]
    return results, best


# --------------------------------------------------------------------------
# entry point
# --------------------------------------------------------------------------

def kernel(x, edge_index, batch, Wg1, bg1, Wg2, bg2, Wg3, bg3, Wr, br,
           Wf, bf, Wo, bo, _time_iters=0):
    in_maps, pad_n, sched = _host_prep(x, edge_index, batch)
    wm = _weight_maps(dict(Wg1=Wg1, bg1=bg1, Wg2=Wg2, bg2=bg2, Wg3=Wg3,
                           bg3=bg3, Wr=Wr, br=br, Wf=Wf, bf=bf, Wo=Wo, bo=bo))
    for m in in_maps:
        m.update(wm)

    key = (pad_n, sched)
    if key not in _cache:
        _cache[key] = _build(pad_n, sched)
    nc = _cache[key]

    results, best = _run(nc, in_maps, time_iters=_time_iters)
    out = np.concatenate([results[c]["out"] for c in range(NCORES)], axis=0)
    if _time_iters:
        return out, best
    return out
